# revision 15
# baseline (speedup 1.0000x reference)
"""DFine multihead attention on 8 Trainium2 NeuronCores (Bass/Tile).

Problem: B=4, S=2048, D=256, H=8, HD=32.
    hp = hidden + pos
    q = hp @ Wq, k = hp @ Wk (per head), v = hidden @ Wv
    scores = (q*HD^-0.5) @ k^T + mask ; attn = softmax(scores)
    out = (attn @ v reshaped) @ Wo + bo

Sharding: core c handles (b = c % 4, head-group hg = c // 4) -> 4 heads each.
Each core returns a partial out (its heads' slice of the D contraction of Wo);
host sums the two head-group partials per batch.

v2: all heavy matmuls in bf16 (1 cyc/row on the PE at N>=256, FWL weight
loads) instead of fp32 (4 cyc/row); softmax denominators fused into the
ctx matmul via a 33rd ones-column on v (M=33 col-tiled matmuls); exp output
in bf16; engine rebalance: ACT does exp only, Pool takes PSUM->SBUF copies
and bias adds, DVE does hp adds / qk bias / normalize.

Kernel structure per core (all on-device):
  1. Load hidden/pos s-tiles, PE-transpose into hiddenT/hpT ([d, s] bf16).
  2. Projections: qT/kT stacks [4h*32, 2048] bf16 (col-packed matmuls),
     vstack in natural [k, (h, 32 v + 1 ones)] bf16 layout.
  3. Main loop over (q-block n of 512, k-tile m of 128, head-pair):
     scores^T = k_m q_n^T via row-packed K=32 bf16 matmuls -> PSUM fp32,
     exp via ScalarE (fused *HD^-0.5) -> SBUF bf16,
     fused ctx^T+denom: [128,33] bf16 lhsT col-tiled at 0/64, PSUM
     accumulate over m (ctx rows 0-31/64-95, denom rows 32/96).
  4. Per q-block: reciprocal of denoms (DVE), partition-broadcast
     (stream_shuffle), normalize ctx^T -> bf16, out-projection matmul
     against Wo slice (bf16), bias add (Pool), DMA out.  The epilogue +
     out-proj are emitted one block late so the PE never stalls.

softmax is computed without max-subtraction: scores here are ~N(0, 4) so
exp() stays well within fp32 range; identical result up to fp rounding.
"""

from contextlib import ExitStack

import numpy as np

import concourse.bass as bass
import concourse.mybir as mybir
import concourse.tile as tile
from concourse import bacc, bass_utils
from concourse.bass import ds, ts
from concourse.masks import make_identity

B, S, D, H = 4, 2048, 256, 8
HD = D // H            # 32
HPG = 4                # heads per group (per core)
HG = H // HPG          # 2 head groups
SCALING = HD ** -0.5
NT = S // 128          # 16 s-tiles
NB = S // 512          # 4 q-blocks
DT = D // 128          # 2 d-tiles
F32 = mybir.dt.float32
BF16 = mybir.dt.bfloat16
N_CORES = 8

_cached = {}


def _build_nc(reps=1):
    nc = bacc.Bacc("TRN2", target_bir_lowering=False, debug=False,
                   num_devices=N_CORES)

    hidden = nc.declare_dram_parameter("hidden", [S, D], F32, isOutput=False).ap()
    pos = nc.declare_dram_parameter("pos", [S, D], F32, isOutput=False).ap()
    wq = nc.declare_dram_parameter("wq", [D, HPG * HD], F32, isOutput=False).ap()
    wk = nc.declare_dram_parameter("wk", [D, HPG * HD], F32, isOutput=False).ap()
    wv = nc.declare_dram_parameter("wv", [D, HPG * HD], F32, isOutput=False).ap()
    bq = nc.declare_dram_parameter("bq", [HPG * HD], F32, isOutput=False).ap()
    bk = nc.declare_dram_parameter("bk", [HPG * HD], F32, isOutput=False).ap()
    bv = nc.declare_dram_parameter("bv", [HPG * HD], F32, isOutput=False).ap()
    wo = nc.declare_dram_parameter("wo", [HPG * HD, D], F32, isOutput=False).ap()
    bo = nc.declare_dram_parameter("bo", [D], F32, isOutput=False).ap()
    out = nc.declare_dram_parameter("out", [S, D], F32, isOutput=True).ap()

    def bcast_dram(ap, n_part):
        # DMA-broadcast a 1D DRAM vector across n_part partitions.
        return bass.AP(tensor=ap.tensor, offset=ap.offset,
                       ap=[[0, n_part]] + list(ap.ap))

    with tile.TileContext(nc) as tc, ExitStack() as stack:
        # ---- persistent SBUF ----
        pers = stack.enter_context(tc.tile_pool(name="persist", bufs=1))
        wq_f = pers.tile([128, DT, HPG * HD], F32, name="wq_f")
        wk_f = pers.tile([128, DT, HPG * HD], F32, name="wk_f")
        wv_f = pers.tile([128, DT, HPG * HD], F32, name="wv_f")
        wq_sb = pers.tile([128, DT, HPG * HD], BF16, name="wq_sb")
        wk_sb = pers.tile([128, DT, HPG * HD], BF16, name="wk_sb")
        wv_sb = pers.tile([128, DT, HPG * HD], BF16, name="wv_sb")
        # wo rearranged to the ctx psum row layout: row 64j+e, col-block
        # `half` holds Wo[32*(2*half+j)+e, :]; rows 32-63/96-127 are zero.
        wo_z = pers.tile([128, HG, D], F32, name="wo_z")
        wo_sb2 = pers.tile([128, HG, D], BF16, name="wo_sb2")
        bq_sb = pers.tile([128, 1], F32, name="bq_sb")
        bk_sb = pers.tile([128, 1], F32, name="bk_sb")
        bv_bc = pers.tile([128, HPG * HD], F32, name="bv_bc")
        bo_bc = pers.tile([128, D], F32, name="bo_bc")
        ident = pers.tile([128, 128], F32, name="ident")
        hiddenT = pers.tile([128, DT, S], BF16, name="hiddenT")
        hpT = pers.tile([128, DT, S], BF16, name="hpT")
        qT = pers.tile([128, S], BF16, name="qT")
        kT = pers.tile([128, S], BF16, name="kT")
        # v in natural [k, (h, 32 v + 1 ones)] layout, per k-tile
        vstack = pers.tile([128, NT, HPG * (HD + 1)], BF16, name="vstack")
        vstack_r = vstack.rearrange("p m (h c) -> p m h c", c=HD + 1)
        # normalized ctx in psum row layout; [ab][half], alternated per block
        ctxn_ab = [[pers.tile([128, 512], BF16, name=f"ctxn{ab}{half}")
                    for half in range(2)] for ab in range(2)]
        rcp_ab = [[pers.tile([128, 512], F32, name=f"rcp{ab}{half}")
                   for half in range(2)] for ab in range(2)]

        # ---- pools (shared between prep and main; PSUM <= 16KB/partition) --
        scp = stack.enter_context(tc.tile_pool(name="scp", bufs=2, space="PSUM"))
        accp = stack.enter_context(tc.tile_pool(name="accp", bufs=1, space="PSUM"))
        outp = stack.enter_context(tc.tile_pool(name="outp", bufs=2, space="PSUM"))
        io = stack.enter_context(tc.tile_pool(name="io", bufs=4))
        sbm = stack.enter_context(tc.tile_pool(name="sbm", bufs=6))
        sbs = stack.enter_context(tc.tile_pool(name="sbs", bufs=2))

        def _body(_iv=None):
            # ---- weights + biases ----
            for dt in range(DT):
                nc.sync.dma_start(out=wq_f[:, dt, :], in_=wq[ts(dt, 128), :])
                nc.sync.dma_start(out=wk_f[:, dt, :], in_=wk[ts(dt, 128), :])
                nc.sync.dma_start(out=wv_f[:, dt, :], in_=wv[ts(dt, 128), :])
            nc.sync.dma_start(out=bq_sb, in_=bq.rearrange("(p one) -> p one", one=1))
            nc.sync.dma_start(out=bk_sb, in_=bk.rearrange("(p one) -> p one", one=1))
            nc.gpsimd.dma_start(out=bv_bc, in_=bcast_dram(bv, 128))
            nc.gpsimd.dma_start(out=bo_bc, in_=bcast_dram(bo, 128))
            make_identity(nc, ident)
            nc.vector.memset(wo_z, 0.0)
            for half in range(HG):
                for j in range(2):
                    h = 2 * half + j
                    nc.sync.dma_start(out=wo_z[ds(64 * j, HD), half, :],
                                      in_=wo[ds(HD * h, HD), :])
            nc.gpsimd.tensor_copy(wq_sb, wq_f)
            nc.gpsimd.tensor_copy(wk_sb, wk_f)
            nc.gpsimd.tensor_copy(wv_sb, wv_f)
            nc.vector.tensor_copy(wo_sb2, wo_z)
            nc.vector.memset(vstack_r[:, :, :, HD:HD + 1], 1.0)

            # ---- transposes: hiddenT (bf16), hpT = hidT + posT (bf16) ----
            for g in range(NT // 4):
                # ps_sc slot 1: 8 hid transposes; slot 2: 8 pos transposes
                ps_trh = scp.tile([128, 1024], F32, name="ps_sc")
                ps_trp = scp.tile([128, 1024], F32, name="ps_sc")
                for j in range(4):
                    m = 4 * g + j
                    hid_t = io.tile([128, D], F32, name="hid_t")
                    nc.sync.dma_start(out=hid_t, in_=hidden[ts(m, 128), :])
                    pos_t = io.tile([128, D], F32, name="pos_t")
                    nc.sync.dma_start(out=pos_t, in_=pos[ts(m, 128), :])
                    for dt in range(DT):
                        nc.tensor.transpose(
                            ps_trh[:, ds(512 * dt + 128 * j, 128)],
                            hid_t[:, ts(dt, 128)], ident)
                        nc.tensor.transpose(
                            ps_trp[:, ds(512 * dt + 128 * j, 128)],
                            pos_t[:, ts(dt, 128)], ident)
                for dt in range(DT):
                    # hiddenT copy on ACT (fp32 PSUM -> bf16 SBUF)
                    nc.scalar.copy(hiddenT[:, dt, ts(g, 512)],
                                   ps_trh[:, ts(dt, 512)])
                    # hpT = posT(PSUM) + hiddenT(SBUF bf16) on DVE
                    nc.vector.tensor_add(hpT[:, dt, ts(g, 512)],
                                         ps_trp[:, ts(dt, 512)],
                                         hiddenT[:, dt, ts(g, 512)])

            # ---- vstack (natural [k, he] bf16 + ones col) ----
            for m in range(NT):
                ps_v = outp.tile([128, D], F32, name="ps_out")
                for dt in range(DT):
                    nc.tensor.matmul(ps_v[:, 0:HPG * HD],
                                     lhsT=hiddenT[:, dt, ts(m, 128)],
                                     rhs=wv_sb[:, dt, :],
                                     start=(dt == 0), stop=(dt == DT - 1))
                nc.vector.tensor_add(
                    vstack_r[:, m, :, 0:HD],
                    ps_v[:, 0:HPG * HD].rearrange("p (h c) -> p h c", c=HD),
                    bv_bc.rearrange("p (h c) -> p h c", c=HD))

            # ---- kT (all blocks), then qT block 0; qT b1-3 deferred ----
            # projections borrow ps_sc slots (safe: scp rotation never holds
            # a live accumulator, unlike the per-block ps_cd tiles)
            def _proj(n, w_sb, b_sb, dest):
                ps_qk = scp.tile([128, 1024], F32, name="ps_sc")
                for h in range(HPG):
                    for dt in range(DT):
                        nc.tensor.matmul(
                            ps_qk[ds(32 * h, 32), 0:512],
                            lhsT=w_sb[:, dt, ds(32 * h, 32)],
                            rhs=hpT[:, dt, ts(n, 512)],
                            start=(dt == 0), stop=(dt == DT - 1),
                            tile_position=(0, 32 * h))
                nc.vector.tensor_scalar(
                    out=dest[:, ts(n, 512)], in0=ps_qk[:, 0:512],
                    scalar1=b_sb, scalar2=None, op0=mybir.AluOpType.add)

            for n in range(NB):
                _proj(n, wk_sb, bk_sb, kT)
            _proj(0, wq_sb, bq_sb, qT)
            deferred_proj = [(n, wq_sb, bq_sb, qT) for n in range(1, NB)]

            # ---- main attention loop ----
            # ctx stays in the psum row layout (ctx rows 64j..64j+31, denom
            # row 64j+32 per head); normalize is partition-aligned, and the
            # out-projection contracts against the zero-padded wo_sb2.
            # Pre-zero the pad rows of the persistent ctxn buffers (and the
            # rcp tiles, whose shuffle input windows span unwritten rows).
            for ab in range(2):
                for half in range(2):
                    for j in range(2):
                        nc.vector.memset(
                            ctxn_ab[ab][half][ds(64 * j + HD, 64 - HD), :], 0.0)
                    nc.vector.memset(rcp_ab[ab][half], 1.0)

            # epilogue of block n-1 is emitted during block n's m-loop
            deferred_epi = []

            def _epilogue(ps_cd, n):
                ctxn = ctxn_ab[n % 2]
                for half in range(2):
                    rcp = rcp_ab[n % 2][half]
                    rbc = sbs.tile([128, 512], F32, name=f"rbc{half}")
                    cn = ctxn[half]
                    for j in range(2):
                        r = ds(64 * j + HD, 1)
                        nc.vector.reciprocal(rcp[r, :], ps_cd[half][r, :])
                        # broadcast den-recip (row 64j+32) onto the head's
                        # ctx rows 64j..64j+31 via offset-base shuffle
                        nc.vector.stream_shuffle(
                            rbc[ds(64 * j, HD), :],
                            rcp[ds(64 * j + HD, HD), :], [0] * HD)
                        nc.vector.tensor_mul(cn[ds(64 * j, HD), :],
                                             ps_cd[half][ds(64 * j, HD), :],
                                             rbc[ds(64 * j, HD), :])
                for st in range(4):
                    ps_o = outp.tile([128, D], F32, name="ps_out")
                    for half in range(2):
                        nc.tensor.matmul(ps_o, lhsT=ctxn[half][:, ts(st, 128)],
                                         rhs=wo_sb2[:, half, :],
                                         start=(half == 0), stop=(half == 1))
                    osb = sbm.tile([128, D], F32, name="osb")
                    nc.vector.tensor_add(osb, ps_o, bo_bc)
                    nc.sync.dma_start(out=out[ds(512 * n + 128 * st, 128), :],
                                      in_=osb)

            for n in range(NB):
                ps_cd = [accp.tile([128, 512], F32, name=f"ps_cd{half}")
                         for half in range(2)]

                def _ctx_den(m, half, expt, ps_cd=ps_cd):
                    # fused ctx^T + denom for one (m, head-pair); one M=33
                    # matmul per head, col-tiled at 0 / 64.  Emitted a few
                    # steps behind the producing exp so the PE never stalls
                    # waiting on ScalarE in program order.
                    for j in range(2):
                        nc.tensor.matmul(
                            ps_cd[half][ds(64 * j, HD + 1), :],
                            lhsT=vstack_r[:, m, 2 * half + j, :],
                            rhs=expt[:, ds(512 * j, 512)],
                            start=(m == 0), stop=(m == NT - 1),
                            tile_position=(0, 64 * j),
                            skip_group_check=True)

                pend = []
                for m in range(NT):
                    for half in range(2):
                        ps_sc = scp.tile([128, 1024], F32, name="ps_sc")
                        for j in range(2):
                            h = 2 * half + j
                            nc.tensor.matmul(
                                ps_sc[:, ds(512 * j, 512)],
                                lhsT=kT[ds(32 * h, 32), ts(m, 128)],
                                rhs=qT[ds(32 * h, 32), ts(n, 512)],
                                start=True, stop=True,
                                tile_position=(32 * h, 0))
                        expt = sbm.tile([128, 1024], BF16, name="expt")
                        nc.scalar.activation(expt, ps_sc,
                                             mybir.ActivationFunctionType.Exp,
                                             scale=SCALING)
                        pend.append((m, half, expt))
                        if len(pend) > 3:
                            _ctx_den(*pend.pop(0))
                        # interleave deferred work early in each block
                        if m == 2 and half == 0:
                            if deferred_epi:
                                _epilogue(*deferred_epi.pop(0))
                        if m in (5, 8, 11) and half == 0 and deferred_proj:
                            _proj(*deferred_proj.pop(0))
                for p in pend:
                    _ctx_den(*p)
                deferred_epi.append((ps_cd, n))

            _epilogue(*deferred_epi.pop(0))

        if reps == 1:
            _body()
        else:
            with tc.For_i(0, reps, 1) as iv:
                _body(iv)
    nc.compile()
    return nc


def _get_nc(reps=1):
    key = f"nc{reps}"
    if key not in _cached:
        _cached[key] = _build_nc(reps)
    return _cached[key]


def _reference_numpy(hidden_states, position_embeddings, attention_mask,
                     Wq, bq, Wk, bk, Wv, bv, Wo, bo):
    # Fallback for nonzero attention_mask (never hit for this problem's spec).
    hp = hidden_states + position_embeddings
    q = np.einsum("bsd,dhe->bshe", hp, Wq) + bq
    k = np.einsum("bsd,dhe->bshe", hp, Wk) + bk
    v = np.einsum("bsd,dhe->bshe", hidden_states, Wv) + bv
    q = q * SCALING
    scores = np.einsum("bqhe,bkhe->bhqk", q, k) + attention_mask[:, None]
    scores -= scores.max(axis=-1, keepdims=True)
    e = np.exp(scores)
    attn = e / e.sum(axis=-1, keepdims=True)
    ctx = np.einsum("bhqk,bkhe->bqhe", attn, v).reshape(B, S, D)
    return (np.einsum("bsd,de->bse", ctx, Wo) + bo).astype(np.float32)


def kernel(hidden_states, position_embeddings, attention_mask,
           Wq, bq, Wk, bk, Wv, bv, Wo, bo, _want_results=False,
           _trace=False, _tmpdir=None):
    args = [np.asarray(a, dtype=np.float32) for a in
            (hidden_states, position_embeddings, attention_mask,
             Wq, bq, Wk, bk, Wv, bv, Wo, bo)]
    (hidden_states, position_embeddings, attention_mask,
     Wq, bq, Wk, bk, Wv, bv, Wo, bo) = args

    if np.any(attention_mask):
        return _reference_numpy(hidden_states, position_embeddings,
                                attention_mask, Wq, bq, Wk, bk, Wv, bv, Wo, bo)

    nc = _get_nc()
    in_maps = []
    for c in range(N_CORES):
        b, hg = c % B, c // B
        hs = slice(hg * HPG, (hg + 1) * HPG)
        in_maps.append({
            "hidden": np.ascontiguousarray(hidden_states[b]),
            "pos": np.ascontiguousarray(position_embeddings[b]),
            "wq": np.ascontiguousarray(Wq[:, hs, :]).reshape(D, HPG * HD),
            "wk": np.ascontiguousarray(Wk[:, hs, :]).reshape(D, HPG * HD),
            "wv": np.ascontiguousarray(Wv[:, hs, :]).reshape(D, HPG * HD),
            "bq": np.ascontiguousarray(bq[hs]).reshape(HPG * HD),
            "bk": np.ascontiguousarray(bk[hs]).reshape(HPG * HD),
            "bv": np.ascontiguousarray(bv[hs]).reshape(HPG * HD),
            "wo": np.ascontiguousarray(Wo[hg * HPG * HD:(hg + 1) * HPG * HD, :]),
            "bo": bo if hg == 0 else np.zeros_like(bo),
        })
    res = bass_utils.run_bass_kernel_spmd(nc, in_maps, list(range(N_CORES)),
                                          trace=_trace, tmpdir=_tmpdir)
    out = np.empty((B, S, D), np.float32)
    for b in range(B):
        out[b] = res.results[b]["out"] + res.results[b + B]["out"]
    if _want_results:
        return out, res
    return out
